# revision 3
# baseline (speedup 1.0000x reference)
"""Trainium2 Bass kernel for nn_CrossAttentionLayer.

Reference computation (per batch element b):
    q = x @ Wq            [N, INNER]   (heads: INNER = H*Dh)
    k = ctx @ Wk          [J, INNER]
    v = ctx @ Wv          [J, INNER]
    sim = q_h @ k_h.T * scale   per head -> softmax over J -> @ v_h
    out = concat_heads @ Wo + bo

Sharding: batch (B=8) across 8 cores, one batch element per core, weights
replicated.  No collectives needed.

Per-core plan (all matmuls bf16 operands, fp32 PSUM accumulation):
  - transpose x -> xT [QD, N], ctx -> ctxT [CD, J] via PE transposes
  - QT [INNER, N] = Wq.T @ xT   (stationary Wq chunks, moving xT)
  - KT [INNER, J] = Wk.T @ ctxT
  - V  [J, INNER] = ctxT.T @ Wv, stored padded per head with a ones column
  - per head h: S^T [J, N] = KT_h.T-contract-d @ QT_h  (K=64 contraction)
      P^T = exp(scale * S^T)  on ACT, written bf16
      O[n, 64+1] = sum_jc P^T_jc.T @ Vpad_h_jc   (ones col -> softmax denom)
      evict O unnormalized (ACT), denom col to den buffer
  - normalize per n-tile: rden = 1/den (DVE), O *= rden (broadcast mul)
  - transpose O -> OT [INNER, N]; out = OT.T @ Wo + bo -> DMA out
"""

import sys

if "/opt/trn_rl_repo" not in sys.path:
    sys.path.insert(0, "/opt/trn_rl_repo")

import numpy as np

import concourse.bass as bass
import concourse.mybir as mybir
import concourse.bacc as bacc
import concourse.tile as tile
from concourse import bass_utils
from concourse.masks import make_identity

P = 128
B, N, J = 8, 2048, 1024
QD, CD, H, Dh = 1024, 768, 16, 64
INNER = H * Dh
NT = N // P      # 16 n tiles
JC = J // P      # 8 context chunks
QC = QD // P     # 8 x-feature chunks
CC = CD // P     # 6 ctx-feature chunks
IC = INNER // P  # 8 inner chunks
NBW = 512        # moving-operand block width
NB = N // NBW    # 4
SCALE = float(Dh) ** -0.5

F32 = mybir.dt.float32
BF16 = mybir.dt.bfloat16
EXP = mybir.ActivationFunctionType.Exp

_CACHE = {}


def _build_module():
    nc = bacc.Bacc("TRN2", target_bir_lowering=False, debug=False)

    x_d = nc.dram_tensor("x", [N, QD], F32, kind="ExternalInput")
    ctx_d = nc.dram_tensor("context", [J, CD], F32, kind="ExternalInput")
    wq_d = nc.dram_tensor("Wq", [QD, INNER], F32, kind="ExternalInput")
    wk_d = nc.dram_tensor("Wk", [CD, INNER], F32, kind="ExternalInput")
    wv_d = nc.dram_tensor("Wv", [CD, INNER], F32, kind="ExternalInput")
    wo_d = nc.dram_tensor("Wo", [INNER, QD], F32, kind="ExternalInput")
    bo_d = nc.dram_tensor("bo", [QD], F32, kind="ExternalInput")
    out_d = nc.dram_tensor("out", [N, QD], F32, kind="ExternalOutput")

    with tile.TileContext(nc) as tc:
        _emit(nc, tc, x_d, ctx_d, wq_d, wk_d, wv_d, wo_d, bo_d, out_d)

    nc.compile()
    return nc


def _emit(nc, tc, x_d, ctx_d, wq_d, wk_d, wv_d, wo_d, bo_d, out_d):
    from contextlib import ExitStack

    est = ExitStack()
    with est:
        # ---------- constants ----------
        const = est.enter_context(tc.tile_pool(name="const", bufs=1))
        identity = const.tile([P, P], BF16, name="identity")
        make_identity(nc, identity)
        ones_row = const.tile([1, P], F32, name="ones_row")
        nc.vector.memset(ones_row[:], 1.0)
        bo_sb = const.tile([1, QD], F32, name="bo_sb")
        nc.sync.dma_start(bo_sb[:], bo_d[:].unsqueeze(0))
        bias_bc = const.tile([P, QD], F32, name="bias_bc")

        with tc.tile_pool(name="cpsum", bufs=2, space="PSUM") as cpsum:
            for qb in range(QD // NBW):
                bp = cpsum.tile([P, NBW], F32, name="bp", tag="bp")
                nc.tensor.matmul(
                    bp[:], ones_row[:, :], bo_sb[:, qb * NBW:(qb + 1) * NBW],
                    start=True, stop=True,
                )
                nc.vector.tensor_copy(bias_bc[:, qb * NBW:(qb + 1) * NBW], bp[:])

        # ---------- persistent activations ----------
        qkv = est.enter_context(tc.tile_pool(name="qkv", bufs=1))
        qt = [qkv.tile([P, N], BF16, name=f"qt{c}", tag=f"qt{c}") for c in range(IC)]
        kt = [qkv.tile([P, J], BF16, name=f"kt{c}", tag=f"kt{c}") for c in range(IC)]
        # v padded: per head 64 cols of V then a ones column (65 per head)
        vp = [qkv.tile([P, H * 65], BF16, name=f"vp{c}", tag=f"vp{c}")
              for c in range(JC)]

        o_bf = est.enter_context(tc.tile_pool(name="o_bf", bufs=1))
        o_sb = o_bf.tile([P, NT * INNER], BF16, name="o_sb")
        den = o_bf.tile([P, NT * H], F32, name="den")
        rden = o_bf.tile([P, NT * H], F32, name="rden")

        wo_pool = est.enter_context(tc.tile_pool(name="wo_pool", bufs=1))
        wo_sb = wo_pool.tile([P, IC * QD], BF16, name="wo_sb")
        for c in range(IC):
            nc.gpsimd.dma_start(
                wo_sb[:, c * QD:(c + 1) * QD], wo_d[c * P:(c + 1) * P, :])

        # ---------- phase A: ctx path (wk, wv, ctxT, KT, Vpad) ----------
        with ExitStack() as actx:
            wkv = actx.enter_context(tc.tile_pool(name="wkv", bufs=1))
            wk_sb = wkv.tile([P, CC * INNER], BF16, name="wk_sb")
            wv_sb = wkv.tile([P, CC * INNER], BF16, name="wv_sb")
            for c in range(CC):
                nc.gpsimd.dma_start(
                    wk_sb[:, c * INNER:(c + 1) * INNER],
                    wk_d[c * P:(c + 1) * P, :])
            for c in range(CC):
                nc.gpsimd.dma_start(
                    wv_sb[:, c * INNER:(c + 1) * INNER],
                    wv_d[c * P:(c + 1) * P, :])

            ctxT_p = actx.enter_context(tc.tile_pool(name="ctxT_p", bufs=1))
            ctxT = [ctxT_p.tile([P, J], BF16, name=f"ctxT{c}", tag=f"ctxT{c}")
                    for c in range(CC)]

            stage = actx.enter_context(tc.tile_pool(name="stage", bufs=3))
            tpsum = actx.enter_context(
                tc.tile_pool(name="tpsum", bufs=2, space="PSUM"))
            ppsum = actx.enter_context(
                tc.tile_pool(name="ppsum", bufs=4, space="PSUM"))

            for jt in range(JC):
                cstage = stage.tile([P, CD], BF16, name="cstage", tag="cstage")
                nc.gpsimd.dma_start(cstage[:], ctx_d[jt * P:(jt + 1) * P, :])
                for cc in range(CC):
                    tp = tpsum.tile([P, P], BF16, name="tp", tag="tp")
                    nc.tensor.transpose(
                        tp[:], cstage[:, cc * P:(cc + 1) * P], identity[:])
                    nc.vector.tensor_copy(
                        ctxT[cc][:, jt * P:(jt + 1) * P], tp[:])

            # KT[ic] [P, J]: stationary Wk chunk, moving ctxT
            for ic in range(IC):
                for jb in range(J // NBW):
                    kp = ppsum.tile([P, NBW], F32, name="kp", tag="pp")
                    for cc in range(CC):
                        nc.tensor.matmul(
                            kp[:],
                            wk_sb[:, cc * INNER + ic * P: cc * INNER + (ic + 1) * P],
                            ctxT[cc][:, jb * NBW:(jb + 1) * NBW],
                            start=(cc == 0), stop=(cc == CC - 1),
                        )
                    nc.vector.tensor_copy(
                        kt[ic][:, jb * NBW:(jb + 1) * NBW], kp[:])

            # V natural [J, INNER] -> padded per head (65 cols per head)
            for jc in range(JC):
                for vb in range(INNER // NBW):
                    vpp = ppsum.tile([P, NBW], F32, name="vpp", tag="pp")
                    for cc in range(CC):
                        nc.tensor.matmul(
                            vpp[:],
                            ctxT[cc][:, jc * P:(jc + 1) * P],
                            wv_sb[:, cc * INNER + vb * NBW: cc * INNER + (vb + 1) * NBW],
                            start=(cc == 0), stop=(cc == CC - 1),
                        )
                    hpb = NBW // Dh  # heads per block = 8
                    dst = vp[jc][:, vb * hpb * 65:(vb + 1) * hpb * 65]
                    dst = dst.rearrange("p (h e) -> p h e", e=65)[:, :, 0:64]
                    src = vpp[:].rearrange("p (h e) -> p h e", e=Dh)
                    nc.vector.tensor_copy(dst, src)
                ones_cols = vp[jc][:].rearrange(
                    "p (h e) -> p h e", e=65)[:, :, 64:65]
                nc.vector.memset(ones_cols, 1.0)

        # ---------- phase B: x path (wq, xT, QT) ----------
        with ExitStack() as bctx:
            wq_pool = bctx.enter_context(tc.tile_pool(name="wq_pool", bufs=1))
            wq_sb = wq_pool.tile([P, QC * INNER], BF16, name="wq_sb")
            for c in range(QC):
                nc.gpsimd.dma_start(
                    wq_sb[:, c * INNER:(c + 1) * INNER],
                    wq_d[c * P:(c + 1) * P, :])

            xT_p = bctx.enter_context(tc.tile_pool(name="xT_p", bufs=1))
            xT = [xT_p.tile([P, N], BF16, name=f"xT{c}", tag=f"xT{c}")
                  for c in range(QC)]

            stage2 = bctx.enter_context(tc.tile_pool(name="stage2", bufs=3))
            tpsum2 = bctx.enter_context(
                tc.tile_pool(name="tpsum2", bufs=2, space="PSUM"))
            ppsum2 = bctx.enter_context(
                tc.tile_pool(name="ppsum2", bufs=4, space="PSUM"))

            for nt in range(NT):
                xstage = stage2.tile([P, QD], BF16, name="xstage", tag="xstage")
                nc.gpsimd.dma_start(xstage[:], x_d[nt * P:(nt + 1) * P, :])
                for qc in range(QC):
                    tp = tpsum2.tile([P, P], BF16, name="tp2", tag="tp2")
                    nc.tensor.transpose(
                        tp[:], xstage[:, qc * P:(qc + 1) * P], identity[:])
                    nc.vector.tensor_copy(xT[qc][:, nt * P:(nt + 1) * P], tp[:])

            for ic in range(IC):
                for nb in range(NB):
                    qp = ppsum2.tile([P, NBW], F32, name="qp", tag="qp2")
                    for qc in range(QC):
                        nc.tensor.matmul(
                            qp[:],
                            wq_sb[:, qc * INNER + ic * P: qc * INNER + (ic + 1) * P],
                            xT[qc][:, nb * NBW:(nb + 1) * NBW],
                            start=(qc == 0), stop=(qc == QC - 1),
                        )
                    nc.vector.tensor_copy(
                        qt[ic][:, nb * NBW:(nb + 1) * NBW], qp[:])

        # ---------- phase C: attention ----------
        with ExitStack() as cctx:
            pt_pool = cctx.enter_context(tc.tile_pool(name="pt_pool", bufs=2))
            spsum = cctx.enter_context(
                tc.tile_pool(name="spsum", bufs=2, space="PSUM"))
            pvpsum = cctx.enter_context(
                tc.tile_pool(name="pvpsum", bufs=4, space="PSUM"))

            for h in range(H):
                ic = h // 2
                po = (h % 2) * Dh
                pts = []
                for jc in range(JC):
                    ptile = pt_pool.tile([P, N], BF16, name=f"pt{jc}",
                                         tag=f"pt{jc}")
                    pts.append(ptile)
                    for half in range(2):
                        sp = spsum.tile([P, 2 * NBW], F32, name="sp", tag="sp")
                        for nbh in range(2):
                            nb = half * 2 + nbh
                            nc.tensor.matmul(
                                sp[:, nbh * NBW:(nbh + 1) * NBW],
                                kt[ic][po:po + Dh, jc * P:(jc + 1) * P],
                                qt[ic][po:po + Dh, nb * NBW:(nb + 1) * NBW],
                                start=True, stop=True,
                            )
                        nc.scalar.activation(
                            ptile[:, half * 2 * NBW:(half + 1) * 2 * NBW],
                            sp[:], EXP, scale=SCALE)

                for nt in range(NT):
                    pv = pvpsum.tile([P, 65], F32, name="pv", tag="pv")
                    for jc in range(JC):
                        nc.tensor.matmul(
                            pv[:],
                            pts[jc][:, nt * P:(nt + 1) * P],
                            vp[jc][:, h * 65: h * 65 + 65],
                            start=(jc == 0), stop=(jc == JC - 1),
                        )
                    nc.scalar.copy(
                        o_sb[:, nt * INNER + h * Dh: nt * INNER + (h + 1) * Dh],
                        pv[:, 0:Dh])
                    nc.vector.tensor_copy(
                        den[:, nt * H + h: nt * H + h + 1], pv[:, 64:65])

            # normalization: per n-tile, all heads at once
            for nt in range(NT):
                nc.vector.reciprocal(
                    rden[:, nt * H:(nt + 1) * H], den[:, nt * H:(nt + 1) * H])
                o_view = o_sb[:, nt * INNER:(nt + 1) * INNER].rearrange(
                    "p (h d) -> p h d", d=Dh)
                r_view = rden[:, nt * H:(nt + 1) * H].unsqueeze(2).broadcast_to(
                    (P, H, Dh))
                nc.vector.tensor_tensor(
                    o_view, o_view, r_view, op=mybir.AluOpType.mult)

        # ---------- phase D: transpose O, output projection ----------
        with ExitStack() as dctx:
            ot_p = dctx.enter_context(tc.tile_pool(name="ot_p", bufs=1))
            ot = [ot_p.tile([P, N], BF16, name=f"ot{c}", tag=f"ot{c}")
                  for c in range(IC)]
            tpsum3 = dctx.enter_context(
                tc.tile_pool(name="tpsum3", bufs=2, space="PSUM"))
            opsum = dctx.enter_context(
                tc.tile_pool(name="opsum", bufs=4, space="PSUM"))
            ostage_p = dctx.enter_context(tc.tile_pool(name="ostage_p", bufs=4))

            for nt in range(NT):
                for ic in range(IC):
                    tp = tpsum3.tile([P, P], BF16, name="tp3", tag="tp3")
                    nc.tensor.transpose(
                        tp[:],
                        o_sb[:, nt * INNER + ic * P: nt * INNER + (ic + 1) * P],
                        identity[:])
                    nc.vector.tensor_copy(ot[ic][:, nt * P:(nt + 1) * P], tp[:])

            for nt in range(NT):
                for qb in range(QD // NBW):
                    op = opsum.tile([P, NBW], F32, name="op", tag="op")
                    for ic in range(IC):
                        nc.tensor.matmul(
                            op[:],
                            ot[ic][:, nt * P:(nt + 1) * P],
                            wo_sb[:, ic * QD + qb * NBW: ic * QD + (qb + 1) * NBW],
                            start=(ic == 0), stop=(ic == IC - 1),
                        )
                    ostage = ostage_p.tile([P, NBW], F32, name="ostage",
                                           tag="ostage")
                    nc.vector.tensor_tensor(
                        ostage[:], op[:], bias_bc[:, qb * NBW:(qb + 1) * NBW],
                        op=mybir.AluOpType.add)
                    nc.sync.dma_start(
                        out_d[nt * P:(nt + 1) * P, qb * NBW:(qb + 1) * NBW],
                        ostage[:])


def _get_module():
    if "nc" not in _CACHE:
        _CACHE["nc"] = _build_module()
    return _CACHE["nc"]


def kernel(x, context, Wq, Wk, Wv, Wo, bo):
    nc = _get_module()
    x = np.asarray(x, dtype=np.float32)
    context = np.asarray(context, dtype=np.float32)
    Wq = np.asarray(Wq, dtype=np.float32)
    Wk = np.asarray(Wk, dtype=np.float32)
    Wv = np.asarray(Wv, dtype=np.float32)
    Wo = np.asarray(Wo, dtype=np.float32)
    bo = np.asarray(bo, dtype=np.float32)

    in_maps = [
        {
            "x": np.ascontiguousarray(x[b]),
            "context": np.ascontiguousarray(context[b]),
            "Wq": Wq, "Wk": Wk, "Wv": Wv, "Wo": Wo, "bo": bo,
        }
        for b in range(B)
    ]
    res = bass_utils.run_bass_kernel_spmd(nc, in_maps, core_ids=list(range(B)))
    return np.stack([res.results[b]["out"] for b in range(B)], axis=0)


if __name__ == "__main__":
    nc = _get_module()
    print("module built and compiled OK")
